# revision 6
# baseline (speedup 1.0000x reference)
"""Trainium2 Bass kernel for batched masked Kabsch-RMSD (Coords2RMSD loss).

4096 samples x 1024 atoms (variable num_atoms), data-parallel over 8
NeuronCores (512 samples/core, 4 tiles of 128 on SBUF partitions).

Per tile: DMA loads x,y (half-row chunks, first tile through the scalar
DMA ring); ACT deinterleaves fp32->bf16 (one op covers x and y per
half); DVE masks (iota fp16 is_lt), applies the mask in one [128,6,1024]
TT, computes all 9 covariance products in one broadcast-AP TT, and folds
15 reduction channels (9 products + ym + xm -> R9/Sy/Sx) through an fp16
binary tree; channel 0 rides an ACT copy+accum to balance engines;
sxx+syy come from a single ACT Square+accum over [xm|ym].

Eigensolve tail works on [128,4] from invariants only: e1=|Rc|^2,
e2=|cof Rc|^2 (cofactors via a cyclic 5x5 replication of Rc, 3 packed
TTs), e3=det^2; the trig solve cos(acos(r)/3) uses a quadratic init + 1
Newton step. Output stored [128,4] per partition (contiguous DMA), host
un-permutes.

GpSimd stays idle: co-running it slows DVE/ACT wide ops up to 5x
(shared SBUF ports, measured). DVE 2x mode needs all operands 2-byte
with packed last AP dim; strided reads drop to 1x.
"""

import math
import numpy as np

import concourse.bass as bass
import concourse.mybir as mybir
from concourse.bass_utils import run_bass_kernel_spmd
from concourse.tile import TileContext

F32 = mybir.dt.float32
BF16 = mybir.dt.bfloat16
FP16 = mybir.dt.float16
I32 = mybir.dt.int32
ALU = mybir.AluOpType
ACT = mybir.ActivationFunctionType

N_CORES = 8
B_FULL = 4096
B_CORE = B_FULL // N_CORES        # 512
N_ATOMS = 1024
ROW = 3 * N_ATOMS                 # 3072
N_TILES = B_CORE // 128           # 4

NEWTON_ITERS = 1
GPS_SX = False      # Sx via one GpSimd tensor_reduce instead of 3 ACT accums
GPS_F4 = False      # fold level 4 + final reduce on GpSimd
SPLIT_LOADS = True


def _build_kernel() -> bass.Bass:
    nc = bass.Bass()

    inp = nc.declare_dram_parameter("input", [B_CORE, ROW], F32, isOutput=False)
    tgt = nc.declare_dram_parameter("target", [B_CORE, ROW], F32, isOutput=False)
    nat = nc.declare_dram_parameter("num_atoms", [B_CORE], I32, isOutput=False)
    out = nc.declare_dram_parameter("out", [B_CORE], F32, isOutput=True)

    with TileContext(nc) as tc:
        with tc.tile_pool(name="p", bufs=1) as pool, \
             tc.tile_pool(name="io", bufs=2) as iop:

            # ---------- one-time setup ----------
            iota_i = pool.tile([128, N_ATOMS], I32, tag="iota_i", name="iota_i")
            nc.gpsimd.iota(iota_i[:], pattern=[[1, N_ATOMS]], base=0,
                           channel_multiplier=0)
            iota_h = pool.tile([128, N_ATOMS], FP16, tag="iota_h", name="iota_h")
            nc.vector.tensor_copy(iota_h[:], iota_i[:])

            n_i = pool.tile([128, N_TILES], I32, tag="n_i", name="n_i")
            nc.sync.dma_start(out=n_i[:],
                              in_=nat[:].rearrange("(t p) -> p t", p=128))
            nf = pool.tile([128, N_TILES], F32, tag="nf", name="nf")
            nc.vector.tensor_copy(nf[:], n_i[:])
            warm = pool.tile([128, 1], F32, tag="warm", name="warm")
            nc.scalar.activation(warm[:], nf[:, 0:1], ACT.Square)
            inv_n = pool.tile([128, N_TILES], F32, tag="inv_n", name="inv_n")
            nc.vector.reciprocal(inv_n[:], nf[:])

            # stats[:, t, 0:9]=R9, 9:12=Sy, 12:15=Sx
            stats = pool.tile([128, N_TILES, 15], F32, tag="stats", name="stats")
            R9 = stats[:, :, 0:9]
            # sse[:, t] = sxx + syy (single merged accumulator)
            sse = pool.tile([128, N_TILES], F32, tag="sse", name="sse")

            act_scr = pool.tile([128, 2 * ROW], BF16, tag="act_scr",
                                name="act_scr")

            # ---------- main loop ----------
            for t in range(N_TILES):
                # xy[:, 0]=y, xy[:, 1]=x -> xyd channels [y(3) | x(3)]
                xy = iop.tile([128, 2, ROW], F32, tag="xy", name="xy")
                xyd = iop.tile([128, 6, N_ATOMS], BF16, tag="xyd", name="xyd")
                r0 = slice(t * 128, (t + 1) * 128)
                H = ROW // 2

                # first tile: x-halves through the scalar-engine DMA ring
                # (free ~3us before the sync ring drains its prolog)
                # tile 0: quarter-granularity chunks so the first deints
                # start as soon as the earliest DMA chunks land; x through
                # the scalar ring, y through sync. later tiles: halves.
                nchunk = 4 if t == 0 else 2
                CW = ROW // nchunk          # fp32 elems per tensor chunk
                AW = N_ATOMS // nchunk      # atoms per chunk
                dmx = nc.scalar if t == 0 else nc.sync
                for q in range(nchunk):
                    dmx.dma_start(out=xy[:, 1, q * CW:(q + 1) * CW],
                                  in_=inp[r0, q * CW:(q + 1) * CW])
                    nc.sync.dma_start(out=xy[:, 0, q * CW:(q + 1) * CW],
                                      in_=tgt[r0, q * CW:(q + 1) * CW])
                for q in range(nchunk):
                    nc.scalar.activation(
                        xyd[:, :, q * AW:(q + 1) * AW]
                        .rearrange("p (u c) n -> p u c n", c=3),
                        xy[:, :, q * CW:(q + 1) * CW]
                        .rearrange("p u (n c) -> p u c n", c=3),
                        ACT.Copy)

                mask = iop.tile([128, N_ATOMS], BF16, tag="mask", name="mask")
                nc.vector.tensor_scalar(mask[:], iota_h[:], nf[:, t : t + 1],
                                        None, ALU.is_lt)

                # red[:, 0:9] products, 9:12 ym, 12:15 xm -- one masked apply
                red = iop.tile([128, 15, N_ATOMS], BF16, tag="red", name="red")
                # tile 0: run apply/products per atom-half so DVE starts
                # right after the first half-deinterleave lands
                chunks = ([(q * 256, (q + 1) * 256) for q in range(4)]
                          if t == 0 else
                          [(0, 512), (512, 1024)] if t == 1
                          else [(0, N_ATOMS)])
                for (a0, a1) in chunks:
                    mb6 = mask[:, a0:a1].rearrange("p n -> p () n") \
                        .broadcast_to((128, 6, a1 - a0))
                    nc.vector.tensor_tensor(red[:, 9:15, a0:a1],
                                            xyd[:, :, a0:a1], mb6, ALU.mult)
                    xmb = red[:, 12:15, a0:a1] \
                        .rearrange("p j n -> p j () n") \
                        .broadcast_to((128, 3, 3, a1 - a0))
                    ymb = red[:, 9:12, a0:a1] \
                        .rearrange("p k n -> p () k n") \
                        .broadcast_to((128, 3, 3, a1 - a0))
                    nc.vector.tensor_tensor(
                        red[:, 0:9, a0:a1].rearrange(
                            "p (j k) n -> p j k n", k=3),
                        xmb, ymb, ALU.mult)

                # fold tree on DVE: channels 1:15 (products[1:9] + ym + xm);
                # channel 0 (R00) via ACT copy+accum to balance engines
                f1 = iop.tile([128, 14, 512], FP16, tag="f1", name="f1")
                nc.vector.tensor_tensor(f1[:], red[:, 1:15, 0:512],
                                        red[:, 1:15, 512:1024], ALU.add)
                f2 = iop.tile([128, 14, 256], FP16, tag="f2", name="f2")
                nc.vector.tensor_tensor(f2[:], f1[:, :, 0:256],
                                        f1[:, :, 256:512], ALU.add)
                f3 = iop.tile([128, 14, 128], FP16, tag="f3", name="f3")
                nc.vector.tensor_tensor(f3[:], f2[:, :, 0:128],
                                        f2[:, :, 128:256], ALU.add)
                f4 = f2[:, :, 0:64]
                nc.vector.tensor_tensor(f4, f3[:, :, 0:64],
                                        f3[:, :, 64:128], ALU.add)
                f5 = f2[:, :, 64:96]
                nc.vector.tensor_tensor(f5, f4[:, :, 0:32],
                                        f4[:, :, 32:64], ALU.add)
                nc.vector.tensor_reduce(stats[:, t, 1:15], f5,
                                        mybir.AxisListType.X, ALU.add)
                nc.scalar.activation(act_scr[:, 0:N_ATOMS], red[:, 0, :],
                                     ACT.Copy, accum_out=stats[:, t, 0:1])

                # sxx+syy in one Square+accum over [xm|ym]
                nc.scalar.activation(
                    act_scr[:],
                    red[:, 9:15, :].rearrange("p c n -> p (c n)"),
                    ACT.Square, accum_out=sse[:, t : t + 1])

            # ---------- invariant eigensolve tail ----------
            T = N_TILES

            def tile4(shape, tag):
                return pool.tile(shape, F32, tag=tag, name=tag)

            v = nc.vector
            s_ = nc.scalar

            W = tile4([128, T, 18], "W")
            Rc = W[:, :, 0:9]
            C = W[:, :, 9:18]
            Rc4 = W[:, :, 0:9].rearrange("p t (j k) -> p t j k", k=3)
            C4 = W[:, :, 9:18].rearrange("p t (j k) -> p t j k", k=3)

            t9a = tile4([128, T, 9], "t9a")
            Sy = stats[:, :, 9:12]
            Sx = stats[:, :, 12:15]
            sxb = Sx.rearrange("p t j -> p t j ()").broadcast_to((128, T, 3, 3))
            syb = Sy.rearrange("p t k -> p t () k").broadcast_to((128, T, 3, 3))
            v.tensor_tensor(t9a[:].rearrange("p t (j k) -> p t j k", k=3),
                            sxb, syb, ALU.mult)
            invb9 = inv_n[:].rearrange("p t -> p t ()").broadcast_to((128, T, 9))
            v.tensor_tensor(t9a[:], t9a[:], invb9, ALU.mult)
            v.tensor_tensor(Rc, R9, t9a[:], ALU.subtract)

            # E55[a, b] = Rc[a%3, b%3]
            E55 = tile4([128, T, 5, 5], "E55")
            v.tensor_copy(E55[:, :, 0:3, 0:3], Rc4)
            v.tensor_copy(E55[:, :, 0:3, 3:5], Rc4[:, :, :, 0:2])
            v.tensor_copy(E55[:, :, 3:5, 0:3], Rc4[:, :, 0:2, :])
            v.tensor_copy(E55[:, :, 3:5, 3:5], Rc4[:, :, 0:2, 0:2])

            v.tensor_tensor(C4, E55[:, :, 1:4, 1:4], E55[:, :, 2:5, 2:5],
                            ALU.mult)
            v.tensor_tensor(t9a[:].rearrange("p t (j k) -> p t j k", k=3),
                            E55[:, :, 1:4, 2:5], E55[:, :, 2:5, 1:4], ALU.mult)
            v.tensor_tensor(C, C, t9a[:], ALU.subtract)

            t3 = tile4([128, T, 3], "t3")
            det = tile4([128, T], "det")
            v.tensor_tensor(t3[:], W[:, :, 0:3], W[:, :, 9:12], ALU.mult)
            v.tensor_reduce(det[:], t3[:], mybir.AxisListType.X, ALU.add)

            sq18 = tile4([128, T, 18], "sq18")
            e12 = tile4([128, T, 2], "e12")
            v.tensor_tensor(sq18[:], W[:], W[:], ALU.mult)
            v.tensor_reduce(e12[:], sq18[:].rearrange("p t (a b) -> p t a b", a=2),
                            mybir.AxisListType.X, ALU.add)
            e1 = e12[:, :, 0]
            e2 = e12[:, :, 1]

            # ee = (sxx+syy) - (|Sx|^2+|Sy|^2)/n
            t6 = tile4([128, T, 6], "t6")
            nrm = tile4([128, T], "nrm")
            ee = tile4([128, T], "ee")
            v.tensor_tensor(t6[:], stats[:, :, 9:15], stats[:, :, 9:15], ALU.mult)
            v.tensor_reduce(nrm[:], t6[:], mybir.AxisListType.X, ALU.add)
            v.tensor_tensor(nrm[:], nrm[:], inv_n[:], ALU.mult)
            v.tensor_tensor(ee[:], sse[:], nrm[:], ALU.subtract)

            u0 = tile4([128, T], "u0")
            u1 = tile4([128, T], "u1")
            q = tile4([128, T], "q")
            pp = tile4([128, T], "pp")
            ip = tile4([128, T], "ip")
            rr = tile4([128, T], "rr")
            cc = tile4([128, T], "cc")

            v.tensor_scalar(q[:], e1, 1.0 / 3.0, None, ALU.mult)
            v.tensor_tensor(u0[:], q[:], q[:], ALU.mult)
            v.scalar_tensor_tensor(u0[:], e2, -1.0 / 3.0, u0[:],
                                   ALU.mult, ALU.add)
            v.tensor_scalar(u0[:], u0[:], 0.0, None, ALU.max)
            s_.activation(pp[:], u0[:], ACT.Sqrt)
            v.tensor_scalar(u0[:], pp[:], 1e-20, None, ALU.max)
            v.reciprocal(ip[:], u0[:])

            detB = tile4([128, T], "detB")
            v.tensor_tensor(u0[:], e1, q[:], ALU.subtract)
            v.tensor_tensor(u0[:], u0[:], q[:], ALU.mult)
            v.tensor_tensor(u0[:], e2, u0[:], ALU.subtract)
            v.tensor_tensor(u0[:], u0[:], q[:], ALU.mult)
            v.tensor_tensor(detB[:], det[:], det[:], ALU.mult)
            v.tensor_tensor(detB[:], detB[:], u0[:], ALU.subtract)

            v.tensor_tensor(u0[:], ip[:], ip[:], ALU.mult)
            v.tensor_tensor(u0[:], u0[:], ip[:], ALU.mult)
            v.tensor_tensor(rr[:], detB[:], u0[:], ALU.mult)
            v.tensor_scalar(rr[:], rr[:], 0.5, 1.0, ALU.mult, ALU.min)
            v.tensor_scalar(rr[:], rr[:], -1.0, None, ALU.max)

            v.tensor_scalar(u0[:], rr[:], -0.116, 0.25, ALU.mult, ALU.add)
            v.tensor_tensor(cc[:], u0[:], rr[:], ALU.mult)
            v.tensor_scalar(cc[:], cc[:], 0.866, None, ALU.add)
            for _ in range(NEWTON_ITERS):
                v.tensor_tensor(u0[:], cc[:], cc[:], ALU.mult)
                v.tensor_scalar(u1[:], u0[:], 12.0, -3.0, ALU.mult, ALU.add)
                v.tensor_tensor(u0[:], u0[:], cc[:], ALU.mult)
                v.scalar_tensor_tensor(u0[:], u0[:], 8.0, rr[:],
                                       ALU.mult, ALU.add)
                v.reciprocal(u1[:], u1[:])
                v.tensor_tensor(cc[:], u0[:], u1[:], ALU.mult)

            sphi = tile4([128, T], "sphi")
            v.tensor_tensor(u0[:], cc[:], cc[:], ALU.mult)
            v.tensor_scalar(u0[:], u0[:], -1.0, 1.0, ALU.mult, ALU.add)
            v.tensor_scalar(u0[:], u0[:], 0.0, None, ALU.max)
            s_.activation(sphi[:], u0[:], ACT.Sqrt)

            lam = tile4([128, T, 3], "lam")
            v.tensor_tensor(u0[:], pp[:], cc[:], ALU.mult)
            v.scalar_tensor_tensor(lam[:, :, 0], u0[:], 2.0, q[:],
                                   ALU.mult, ALU.add)
            v.scalar_tensor_tensor(u1[:], u0[:], -1.0, q[:],
                                   ALU.mult, ALU.add)
            v.tensor_tensor(u0[:], pp[:], sphi[:], ALU.mult)
            v.scalar_tensor_tensor(lam[:, :, 2], u0[:], -math.sqrt(3.0), u1[:],
                                   ALU.mult, ALU.add)
            v.scalar_tensor_tensor(u0[:], q[:], 3.0, lam[:, :, 0],
                                   ALU.mult, ALU.subtract)
            v.tensor_tensor(lam[:, :, 1], u0[:], lam[:, :, 2], ALU.subtract)

            v.tensor_scalar(lam[:], lam[:], 0.0, None, ALU.max)
            s_.activation(lam[:], lam[:], ACT.Sqrt)

            dsign = tile4([128, T], "dsign")
            v.tensor_scalar(dsign[:], det[:], 0.0, None, ALU.is_ge)
            v.tensor_scalar(dsign[:], dsign[:], 2.0, -1.0, ALU.mult, ALU.add)

            ssum = tile4([128, T], "ssum")
            v.tensor_tensor(u0[:], dsign[:], lam[:, :, 2], ALU.mult)
            v.tensor_tensor(ssum[:], lam[:, :, 0], lam[:, :, 1], ALU.add)
            v.tensor_tensor(ssum[:], ssum[:], u0[:], ALU.add)

            res = tile4([128, T], "res")
            v.scalar_tensor_tensor(res[:], ssum[:], -2.0, ee[:],
                                   ALU.mult, ALU.add)
            v.tensor_tensor(res[:], res[:], inv_n[:], ALU.mult)
            v.tensor_scalar(res[:], res[:], 1e-12, None, ALU.max)
            s_.activation(res[:], res[:], ACT.Sqrt)

            # [128,4] per-partition rows -> contiguous 16B writes; host
            # un-permutes (out[p*4+t] = res[p,t])
            nc.sync.dma_start(
                out=out[:].rearrange("(p t) -> p t", p=128),
                in_=res[:])

    _split_multi_waits(nc)
    return nc


def _split_multi_waits(nc):
    ctr = 0
    for f in nc.m.functions:
        for bb in f.blocks:
            new = []
            for inst in bb.instructions:
                si = inst.sync_info
                if si is not None and si.on_wait and len(si.on_wait) > 1:
                    waits = list(si.on_wait)
                    for w in waits[:-1]:
                        ctr += 1
                        new.append(mybir.InstNoOp(
                            name=f"waitnop-{ctr}", engine=inst.engine,
                            ins=[], outs=[],
                            sync_info=mybir.SyncInfo(on_wait=[w],
                                                     on_update=[])))
                    inst.sync_info = mybir.SyncInfo(on_wait=[waits[-1]],
                                                    on_update=si.on_update)
                new.append(inst)
            bb.instructions = new


_NC_CACHE = None


def _get_nc():
    global _NC_CACHE
    if _NC_CACHE is None:
        _NC_CACHE = _build_kernel()
    return _NC_CACHE


def kernel(input: np.ndarray, target: np.ndarray, num_atoms: np.ndarray,
           **_unused) -> np.ndarray:
    input = np.ascontiguousarray(np.asarray(input, dtype=np.float32))
    target = np.ascontiguousarray(np.asarray(target, dtype=np.float32))
    num_atoms = np.ascontiguousarray(np.asarray(num_atoms, dtype=np.int32))

    nc = _get_nc()
    in_maps = []
    for i in range(N_CORES):
        sl = slice(i * B_CORE, (i + 1) * B_CORE)
        in_maps.append({
            "input": input[sl],
            "target": target[sl],
            "num_atoms": num_atoms[sl],
        })
    res = run_bass_kernel_spmd(nc, in_maps, list(range(N_CORES)))
    outs = [res.results[i]["out"].reshape(128, N_TILES).T.ravel()
            for i in range(N_CORES)]
    return np.concatenate(outs).astype(np.float32)


if __name__ == "__main__":
    rng = np.random.default_rng(0)
    inp = rng.standard_normal((B_FULL, ROW), dtype=np.float32)
    tgt = rng.standard_normal((B_FULL, ROW), dtype=np.float32)
    na = rng.integers(8, N_ATOMS + 1, size=(B_FULL,), dtype=np.int32)
    print(kernel(input=inp, target=tgt, num_atoms=na)[:8])
